# revision 39
# baseline (speedup 1.0000x reference)
"""Multi-head self-attention (B=2, S=2048, D=1024, H=16) on 8 TRN2 NeuronCores.

Sharding: data-parallel over batch (2) x tensor-parallel over head-groups (4).
Core c = b*4 + hg handles batch b, heads hg*4..hg*4+3 (4 heads, 256 features).

Per-core device program (SPMD, identical on all cores):
  - QKV projections for the core's 256 output features (column-parallel)
  - full S x S attention for its 4 heads (softmax without max-subtraction,
    denominators via an appended ones-column in the PV matmul)
  - partial output projection (row-parallel), written as [128, 8, 2048]
Host: shards/relayouts inputs, sums the 4 partial outputs per batch
(the "all-reduce"), adds bo, and untransposes.

Schedule notes: the scalar-engine exp of the S x S scores (~134us busy) and
the PE matmul stream (~136us) are co-pacers. All projections, the V pass and
the output projections are woven into the ACT-paced attention steps; each
step's PV interleaves with the next step's scores. Inputs are uploaded
pre-sliced into per-piece contiguous tensors so each DMA is 128 large
descriptors (the v1 column-sliced loads generated 16k 1KB descriptors and
made the input stream descriptor-rate-bound), chained in priority order so
the first scores chunk can start at ~8us.
"""

import numpy as np

B, S, D = 2, 2048, 1024
H, DK = 16, 64
NCORES = 8
HG = 4          # head groups (tensor parallel)
HPG = 4         # heads per group
F = HPG * DK    # 256 local features per core
SCALE = 1.0 / np.sqrt(DK)

_compiled = {}


def _build():
    import concourse.bacc as bacc
    import concourse.tile as tile
    from concourse import mybir

    f32 = mybir.dt.float32
    bf16 = mybir.dt.bfloat16
    Exp = mybir.ActivationFunctionType.Exp
    mult = mybir.AluOpType.mult

    nc = bacc.Bacc("TRN2", target_bir_lowering=False, debug=False,
                   enable_asserts=True, num_devices=NCORES)

    NDT = D // 128   # 8 d-tiles
    NST = S // 128   # 16 s-tiles (j tiles)
    NSB = S // 512   # 4 s-blocks (i blocks)

    # inputs pre-relayouted on host: x pieces are [128, 8(dt), cols] with the
    # partition dim first so one DMA is 128 contiguous descriptors.
    XQW = [256, 256, 512, 1024]
    XKW = [256, 256, 512, 512, 512]
    xqd = [nc.dram_tensor(f"xq{i}", (128, NDT, w), bf16, kind="ExternalInput")
           for i, w in enumerate(XQW)]
    xkd = [nc.dram_tensor(f"xk{i}", (128, NDT, w), bf16, kind="ExternalInput")
           for i, w in enumerate(XKW)]
    xvd = [nc.dram_tensor(f"xv{i}", shp, bf16, kind="ExternalInput")
           for i, shp in enumerate([(128, NDT, 1024), (128, NDT, 1024)])]
    wq = nc.dram_tensor("wq", (128, NDT, F), bf16, kind="ExternalInput")
    wk = nc.dram_tensor("wk", (128, NDT, F), bf16, kind="ExternalInput")
    wv = nc.dram_tensor("wv", (128, NDT, F), bf16, kind="ExternalInput")
    wo = nc.dram_tensor("wo", (128, 2, D), bf16, kind="ExternalInput")
    bq = nc.dram_tensor("bq", (128, 2), f32, kind="ExternalInput")  # bias, f-tiled
    bk = nc.dram_tensor("bk", (128, 2), f32, kind="ExternalInput")
    bv = nc.dram_tensor("bv", (1, F), f32, kind="ExternalInput")
    out = nc.dram_tensor("out", (128, NSB, NDT, 512), bf16, kind="ExternalOutput")

    with tile.TileContext(nc) as tc:
        import contextlib
        with contextlib.ExitStack() as ctx:
            consts = ctx.enter_context(tc.tile_pool(name="consts", bufs=1))
            atp = ctx.enter_context(tc.tile_pool(name="atp", bufs=10))
            acts = ctx.enter_context(tc.tile_pool(name="acts", bufs=1))
            ostage = ctx.enter_context(tc.tile_pool(name="ostage", bufs=3))
            small = ctx.enter_context(tc.tile_pool(name="small", bufs=1))
            ps = ctx.enter_context(tc.tile_pool(name="ps", bufs=1, space="PSUM"))

            # ---- resident input/weight tiles (one per dram piece) ----
            xq_sb = [acts.tile([128, NDT, w], bf16, tag=f"xq{i}", name=f"xq{i}")
                     for i, w in enumerate(XQW)]
            xk_sb = [acts.tile([128, NDT, w], bf16, tag=f"xk{i}", name=f"xk{i}")
                     for i, w in enumerate(XKW)]
            xv_sb = [acts.tile([128, NDT, 1024], bf16, tag=f"xv{i}", name=f"xv{i}")
                     for i in range(2)]

            def x_ap(ts, dt, c0, w):
                # columns [c0, c0+w) of d-tile dt across the piece tiles
                off = 0
                for t in ts:
                    n = t.shape[2]
                    if c0 < off + n:
                        return t[:, dt, c0 - off:c0 - off + w]
                    off += n
                raise AssertionError

            def xv_ap(dt, st):
                t, r = divmod(st, 8)
                return xv_sb[t][:, dt, r * 128:(r + 1) * 128]

            wq_sb = consts.tile([128, NDT, F], bf16, tag="wq")
            wk_sb = consts.tile([128, NDT, F], bf16, tag="wk")
            wv_sb = consts.tile([128, NDT, F], bf16, tag="wv")
            wo_sb = consts.tile([128, 2, D], bf16, tag="wo")
            bq_sb = consts.tile([128, 2], f32, tag="bq")
            bk_sb = consts.tile([128, 2], f32, tag="bk")
            bv_sb = consts.tile([128, F], f32, tag="bv")

            # persistent activations
            qh_t = [[acts.tile([128, 512], bf16, tag=f"qh{ft}{sb}", name=f"qh{ft}{sb}")
                     for sb in range(NSB)] for ft in range(2)]
            kh_t = [[acts.tile([128, 512], bf16, tag=f"kh{ft}{sb}", name=f"kh{ft}{sb}")
                     for sb in range(NSB)] for ft in range(2)]
            # vh: [s, h, c+1] with ones column at c=64 (PV denominator trick)
            vh_t = [acts.tile([128, HPG, DK + 1], bf16, tag=f"vh{st}", name=f"vh{st}")
                    for st in range(NST)]
            for st in range(NST):
                nc.vector.memset(vh_t[st][:, :, DK:DK + 1], 1.0)
            # y: normalized attention output, [f, s] per (ft, sb)
            y_t = [[acts.tile([128, 512], bf16, tag=f"y{ft}{sb}", name=f"y{ft}{sb}")
                    for sb in range(NSB)] for ft in range(2)]

            # ---- input DMAs: whole-piece transfers issued in priority
            # order (queues process descriptors FIFO, so issue order is
            # arrival order; explicit dep-chaining costs ~5us per link) ----
            def dma_in(dst, src):
                return nc.sync.dma_start(dst, src)

            dma_in(wk_sb[:], wk.ap())
            dma_in(xk_sb[0][:], xkd[0].ap())
            dma_in(wq_sb[:], wq.ap())
            dma_in(xq_sb[0][:], xqd[0].ap())
            nc.sync.dma_start(bk_sb[:], bk.ap()[:])
            nc.sync.dma_start(bq_sb[:], bq.ap()[:])
            dma_in(xk_sb[1][:], xkd[1].ap())
            dma_in(xq_sb[1][:], xqd[1].ap())
            nc.sync.dma_start(bv_sb[:], bv.ap().to_broadcast((128, F)))
            dma_in(xk_sb[2][:], xkd[2].ap())
            dma_in(xk_sb[3][:], xkd[3].ap())
            dma_in(xk_sb[4][:], xkd[4].ap())
            dma_in(wv_sb[:], wv.ap())
            dma_in(xv_sb[0][:], xvd[0].ap())
            dma_in(xq_sb[2][:], xqd[2].ap())
            dma_in(xv_sb[1][:], xvd[1].ap())
            dma_in(xq_sb[3][:], xqd[3].ap())
            dma_in(wo_sb[:], wo.ap())

            # ---- PE warmup: keep HAM at 8/8 through the DMA lead-in so the
            # first projections run at 2.4 GHz instead of 1.2 ----
            wupa = consts.tile([128, 32], bf16, tag="wup")
            nc.vector.memset(wupa[:], 0.0)
            wups = ps.tile([128, 512], f32, tag="w1", bufs=4, name="wups")

            def warm(n):
                for _ in range(n):
                    nc.tensor.matmul(wups[0:32, 0:32], wupa[:, 0:32],
                                     wupa[:, 0:32], start=True, stop=True)
            warm(150)

            # ---- background PE units (woven into the attention steps) ----
            _uid = [0]

            def proj_unit(w_sb, b_sb, xts, dst, ft, sb, off=0, w=512):
                _uid[0] += 1
                acc = ps.tile([128, 512], f32, tag="w1", bufs=4,
                              name=f"pj{_uid[0]}")
                # split the column window at x-piece boundaries
                bounds, c = [], 0
                for t in xts:
                    c += t.shape[2]
                    bounds.append(c)
                c0, wins = sb * 512 + off, []
                end = c0 + w
                while c0 < end:
                    nxt = min([b for b in bounds if b > c0] + [end])
                    wins.append((c0, min(nxt, end)))
                    c0 = min(nxt, end)
                for (a, b_) in wins:
                    oa = a - (sb * 512)
                    for dt in range(NDT):
                        nc.tensor.matmul(
                            acc[:, oa:oa + b_ - a],
                            w_sb[:, dt, ft * 128:(ft + 1) * 128],
                            x_ap(xts, dt, a, b_ - a),
                            start=(dt == 0), stop=(dt == NDT - 1),
                        )
                nc.vector.tensor_scalar_add(dst[ft][sb][:, off:off + w],
                                            acc[:, off:off + w],
                                            b_sb[:, ft:ft + 1])

            def v_unit(g):
                accs = [ps.tile([128, 512], f32, tag="w1", bufs=4,
                                name=f"vps{g}{j}") for j in range(2)]
                for dt in range(NDT):
                    for j in range(2):
                        st = 2 * g + j
                        nc.tensor.matmul(
                            accs[j][:, 0:F],
                            xv_ap(dt, st),
                            wv_sb[:, dt, :],
                            start=(dt == 0), stop=(dt == NDT - 1),
                        )
                for j in range(2):
                    st = 2 * g + j
                    nc.vector.tensor_tensor(
                        vh_t[st][:, :, 0:DK],
                        accs[j][:, 0:F].rearrange("p (h c) -> p h c", h=HPG),
                        bv_sb[:].rearrange("p (h c) -> p h c", h=HPG),
                        mybir.AluOpType.add,
                    )

            # ---- attention chunk helpers (per (ft, ib) step, jc chunks) ----
            def scores_chunk(pr, ib, jc, halves=None):
                # halves: list of (q0, q1) column windows; each gets its own
                # exp call (used to start ACT before the full qh is ready)
                ft = pr
                at = atp.tile([128, 4, 512], bf16, tag="at", name=f"at{pr}{ib}{jc}")
                for jj in range(2):
                    jt = jc * 2 + jj
                    sc = ps.tile([128, 2, 512], f32, tag="w2", bufs=2, name="sc")
                    for (q0, q1) in (halves or [(0, 512)]):
                        for hh in range(2):
                            base = hh * 64
                            nc.tensor.matmul(
                                sc[:, hh, q0:q1],
                                kh_t[ft][jt // 4][base:base + 64,
                                                  (jt % 4) * 128:(jt % 4 + 1) * 128],
                                qh_t[ft][ib][base:base + 64, q0:q1],
                                start=True, stop=True,
                                tile_position=(base, 0),
                            )
                        nc.scalar.activation(
                            at[:, jj * 2:jj * 2 + 2, q0:q1],
                            sc[:, :, q0:q1],
                            Exp, scale=float(SCALE),
                        )
                return at

            def pv_chunk(pr, pv_ps, at, jc):
                for hh in range(2):
                    h = 2 * pr + hh
                    for jj in range(2):
                        jt = 2 * jc + jj
                        nc.tensor.matmul(
                            pv_ps[hh][0:DK + 1, :],
                            vh_t[jt][:, h, :],
                            at[:, 2 * jj + hh, :],
                            start=(jt == 0), stop=(jt == NST - 1),
                        )

            def finish_ib(pr, ib, pv_ps):
                # hh0/hh1 chains interleaved so DVE and GPSIMD pipeline
                ft = pr
                dens, recs, rbs = [], [], []
                for hh in range(2):
                    den = small.tile([1, 512], f32, tag=f"den{hh}")
                    nc.vector.tensor_copy(den[:], pv_ps[hh][DK:DK + 1, :])
                    dens.append(den)
                for hh in range(2):
                    rec = small.tile([1, 512], f32, tag=f"rec{hh}")
                    nc.vector.reciprocal_approx_fast(rec[:], dens[hh][:])
                    recs.append(rec)
                for hh in range(2):
                    rb = small.tile([64, 512], f32, tag=f"rb{hh}")
                    nc.gpsimd.partition_broadcast(rb[:], recs[hh][:])
                    rbs.append(rb)
                for hh in range(2):
                    nc.vector.tensor_tensor(
                        y_t[ft][ib][hh * 64:hh * 64 + 64, :],
                        pv_ps[hh][0:DK, :],
                        rbs[hh][:],
                        mult,
                    )

            # ---- output projection: one unit = 2 et tiles + 1 out DMA ----
            def outproj_unit(sb, ep):
                o_sb = ostage.tile([128, 2, 512], bf16, tag="ost",
                                   name=f"os{ep}{sb}")
                for i in range(2):
                    et = 2 * ep + i
                    po = ps.tile([128, 512], f32, tag="w1", bufs=4,
                                 name=f"po{et}{sb}")
                    for ft in range(2):
                        nc.tensor.matmul(
                            po[:],
                            wo_sb[:, ft, et * 128:(et + 1) * 128],
                            y_t[ft][sb][:],
                            start=(ft == 0), stop=(ft == 1),
                        )
                    nc.vector.tensor_copy(o_sb[:, i, :], po[:])
                nc.sync.dma_start(
                    out.ap()[:, sb, 2 * ep:2 * ep + 2, :],
                    o_sb[:],
                )

            # ---- the woven schedule ----
            # stage 1: half-width lead-in — kh/qh for queries/keys 0:256
            # only (the 256-wide first DMA pieces), so the PE starts ~5us
            # earlier; the first two scores chunks are emitted query-split.
            proj_unit(wk_sb, bk_sb, xk_sb, kh_t, 0, 0, 0, 256)
            warm(80)   # bridge the xq0a DMA wait so HAM stays at 8/8
            proj_unit(wq_sb, bq_sb, xq_sb, qh_t, 0, 0, 0, 256)
            warm(40)

            def first_chunk():
                # jc0 of step (0,0), query-split so scoring starts on the
                # 256-wide lead DMA pieces; the 256:512 projections are
                # emitted between the two halves (PE queue is in-order).
                at = atp.tile([128, 4, 512], bf16, tag="at", name="at000")
                scs = [ps.tile([128, 2, 512], f32, tag="w2", bufs=2,
                               name="sc") for _ in range(2)]

                def mmhalf(jj, q0, q1):
                    for hh in range(2):
                        base = hh * 64
                        nc.tensor.matmul(
                            scs[jj][:, hh, q0:q1],
                            kh_t[0][0][base:base + 64, jj * 128:(jj + 1) * 128],
                            qh_t[0][0][base:base + 64, q0:q1],
                            start=True, stop=True,
                            tile_position=(base, 0),
                        )
                    nc.scalar.activation(
                        at[:, jj * 2:jj * 2 + 2, q0:q1],
                        scs[jj][:, :, q0:q1],
                        Exp, scale=float(SCALE),
                    )
                mmhalf(0, 0, 256)
                mmhalf(1, 0, 256)
                proj_unit(wq_sb, bq_sb, xq_sb, qh_t, 0, 0, 256, 256)
                mmhalf(0, 256, 512)
                mmhalf(1, 256, 512)
                proj_unit(wk_sb, bk_sb, xk_sb, kh_t, 0, 0, 256, 256)
                return at

            # background units per (step, jc-slot), placed after both their
            # DMA piece lands (see issue order) and before their consumer.
            # v(g) feeds pv chunk g of the NEXT step; kh(0,sb) feeds scores
            # jc=2sb of step 0; qh(ft,ib) feeds step (4ft+ib)'s scores;
            # op(sb,ep) = output projection units, after finish of (1,sb).
            slots = {
                (0, 1): [("kh", 0, 1)],
                (0, 3): [("kh", 0, 2)],
                (0, 4): [("kh", 0, 3)],
                (0, 5): [("v", 0)],
                (0, 6): [("v", 1)],
                (0, 7): [("qh", 0, 1)],
                (1, 0): [("v", 2)],
                (1, 1): [("v", 3)],
                (1, 2): [("v", 4)],
                (1, 3): [("v", 5)],
                (1, 4): [("v", 6)],
                (1, 5): [("v", 7)],
                (1, 7): [("qh", 0, 2)],
                (2, 1): [("kh", 1, 0)],
                (2, 2): [("qh", 0, 3)],
                (2, 4): [("kh", 1, 1)],
                (3, 1): [("kh", 1, 2)],
                (3, 2): [("kh", 1, 3)],
                (3, 4): [("qh", 1, 0)],
                (4, 1): [("qh", 1, 1)],
                (5, 1): [("qh", 1, 2)],
                (6, 1): [("qh", 1, 3)],
                (6, 2): [("op", 0, 0)], (6, 3): [("op", 0, 1)],
                (6, 4): [("op", 0, 2)], (6, 5): [("op", 0, 3)],
                (7, 2): [("op", 1, 0)], (7, 3): [("op", 1, 1)],
                (7, 4): [("op", 1, 2)], (7, 5): [("op", 1, 3)],
            }

            def run_unit(u):
                if u[0] == "kh":
                    proj_unit(wk_sb, bk_sb, xk_sb, kh_t, u[1], u[2])
                elif u[0] == "qh":
                    proj_unit(wq_sb, bq_sb, xq_sb, qh_t, u[1], u[2])
                elif u[0] == "v":
                    v_unit(u[1])
                elif u[0] == "op":
                    outproj_unit(u[1], u[2])

            # software pipeline: the previous step's pv chunk jc-1 runs at
            # slot jc; its last chunk + finish run after the NEXT step's
            # first scores chunk so the step boundary never blocks ACT.
            seq = [(0, 0), (0, 1), (0, 2), (0, 3), (1, 0), (1, 1), (1, 2), (1, 3)]
            prev = None   # (pr, ib, pv_ps, at_list) - one step behind
            prev2 = None  # two steps behind, needs last chunk + finish
            for si, (pr, ib) in enumerate(seq):
                at_list = []
                ppv = None
                for jc in range(NST // 2):
                    if si == 0 and jc == 0:
                        at_list.append(first_chunk())
                        continue
                    at_list.append(scores_chunk(pr, ib, jc))
                    if jc == 0:
                        if prev2 is not None:
                            pv_chunk(prev2[0], prev2[2], prev2[3][7], 7)
                            finish_ib(prev2[0], prev2[1], prev2[2])
                    else:
                        if prev is not None:
                            if jc == 1:
                                ppv = [ps.tile([128, 512], f32, tag="w1", bufs=4,
                                               name=f"pv{prev[0]}{prev[1]}_{i}")
                                       for i in range(2)]
                                prev = (prev[0], prev[1], ppv, prev[3])
                            pv_chunk(prev[0], prev[2], prev[3][jc - 1], jc - 1)
                    for u in slots.get((si, jc), ()):
                        run_unit(u)
                prev2 = prev
                prev = (pr, ib, None, at_list)

            # tail: finish step 6's pipeline, then drain step 7's PV with the
            # sb2 output projection interleaved; finishes overlap on DVE.
            pv_chunk(prev2[0], prev2[2], prev2[3][7], 7)
            finish_ib(prev2[0], prev2[1], prev2[2])
            lpv = [ps.tile([128, 512], f32, tag="w1", bufs=4, name=f"pvlast{i}")
                   for i in range(2)]
            for jc in range(NST // 2):
                pv_chunk(prev[0], lpv, prev[3][jc], jc)
                if jc >= 4:
                    outproj_unit(2, jc - 4)   # y[*][2] ready via finish above
            finish_ib(prev[0], prev[1], lpv)   # DVE, overlaps outproj(2) PE
            warm(60)   # bridge the finish chain so outproj(3) stays at 2.4GHz
            for ep in range(4):
                outproj_unit(3, ep)

    nc.compile()
    return nc


def _get_nc():
    if "nc" not in _compiled:
        _compiled["nc"] = _build()
    return _compiled["nc"]


def kernel(q, k, v, Wq, bq, Wk, bk, Wv, bv, Wo, bo):
    outp, _ = _run(q, k, v, Wq, bq, Wk, bk, Wv, bv, Wo, bo)
    return outp


def _x_pieces(xT, bf, widths):
    x3 = np.transpose(xT.reshape(8, 128, S), (1, 0, 2))  # [128, 8, S]
    out, c = [], 0
    for w in widths:
        out.append(np.ascontiguousarray(x3[:, :, c:c + w]).astype(bf))
        c += w
    return out


def _v_pieces(xT, bf):
    x3 = np.transpose(xT.reshape(8, 128, S), (1, 0, 2))
    return [np.ascontiguousarray(x3[:, :, 0:1024]).astype(bf),
            np.ascontiguousarray(x3[:, :, 1024:2048]).astype(bf)]


def _w_relayout(wT, bf):
    # wT: [D, F] -> [128, 8, F]
    return np.ascontiguousarray(
        np.transpose(wT.reshape(8, 128, F), (1, 0, 2))).astype(bf)


def _run(q, k, v, Wq, bq, Wk, bk, Wv, bv, Wo, bo, **run_kwargs):
    from concourse.bass_utils import run_bass_kernel_spmd

    nc = _get_nc()

    q = np.asarray(q, np.float32)
    k = np.asarray(k, np.float32)
    v = np.asarray(v, np.float32)
    Wq = np.asarray(Wq, np.float32)
    Wk = np.asarray(Wk, np.float32)
    Wv = np.asarray(Wv, np.float32)
    Wo = np.asarray(Wo, np.float32)
    bq = np.asarray(bq, np.float32)
    bk = np.asarray(bk, np.float32)
    bv = np.asarray(bv, np.float32)
    bo = np.asarray(bo, np.float32)

    import ml_dtypes
    bf = ml_dtypes.bfloat16
    xqP = [_x_pieces(np.ascontiguousarray(q[b].T), bf, [256, 256, 512, 1024]) for b in range(B)]
    xkP = [_x_pieces(np.ascontiguousarray(k[b].T), bf, [256, 256, 512, 512, 512]) for b in range(B)]
    xvP = [_v_pieces(np.ascontiguousarray(v[b].T), bf) for b in range(B)]

    in_maps = []
    for c in range(NCORES):
        b, hg = divmod(c, HG)
        rows = slice(hg * F, (hg + 1) * F)
        woT = np.ascontiguousarray(Wo[:, rows].T)  # [F, D]
        wo_r = np.ascontiguousarray(
            np.transpose(woT.reshape(2, 128, D), (1, 0, 2))).astype(bf)
        m = {
            "wq": _w_relayout(np.ascontiguousarray(Wq[rows].T), bf),
            "wk": _w_relayout(np.ascontiguousarray(Wk[rows].T), bf),
            "wv": _w_relayout(np.ascontiguousarray(Wv[rows].T), bf),
            "wo": wo_r,
            "bq": np.ascontiguousarray(bq[rows].reshape(2, 128).T),
            "bk": np.ascontiguousarray(bk[rows].reshape(2, 128).T),
            "bv": np.ascontiguousarray(bv[rows].reshape(1, F)),
        }
        for i in range(4):
            m[f"xq{i}"] = xqP[b][i]
        for i in range(5):
            m[f"xk{i}"] = xkP[b][i]
        for i in range(2):
            m[f"xv{i}"] = xvP[b][i]
        in_maps.append(m)

    res = run_bass_kernel_spmd(nc, in_maps, core_ids=list(range(NCORES)), **run_kwargs)

    outp = np.empty((B, S, D), np.float32)
    for b in range(B):
        acc = res.results[b * HG]["out"].astype(np.float32)
        for hg in range(1, HG):
            acc = acc + res.results[b * HG + hg]["out"].astype(np.float32)
        # [128, 4, 8, 512] -> [D, S] -> [S, D]
        full = np.transpose(acc, (2, 0, 1, 3)).reshape(D, S)
        outp[b] = full.T + bo[None, :]
    return outp, res


# revision 40
# speedup vs baseline: 1.0354x; 1.0354x over previous
"""Multi-head self-attention (B=2, S=2048, D=1024, H=16) on 8 TRN2 NeuronCores.

Sharding: data-parallel over batch (2) x tensor-parallel over head-groups (4).
Core c = b*4 + hg handles batch b, heads hg*4..hg*4+3 (4 heads, 256 features).

Per-core device program (SPMD, identical on all cores):
  - QKV projections for the core's 256 output features (column-parallel)
  - full S x S attention for its 4 heads (softmax without max-subtraction,
    denominators via an appended ones-column in the PV matmul)
  - partial output projection (row-parallel), written as [128, 8, 2048]
Host: shards/relayouts inputs, sums the 4 partial outputs per batch
(the "all-reduce"), adds bo, and untransposes.

Schedule notes: the scalar-engine exp of the S x S scores (~134us busy) and
the PE matmul stream (~136us) are co-pacers. All projections, the V pass and
the output projections are woven into the ACT-paced attention steps; each
step's PV interleaves with the next step's scores. Inputs are uploaded
pre-sliced into per-piece contiguous tensors so each DMA is 128 large
descriptors (the v1 column-sliced loads generated 16k 1KB descriptors and
made the input stream descriptor-rate-bound), chained in priority order so
the first scores chunk can start at ~8us.
"""

import numpy as np

B, S, D = 2, 2048, 1024
H, DK = 16, 64
NCORES = 8
HG = 4          # head groups (tensor parallel)
HPG = 4         # heads per group
F = HPG * DK    # 256 local features per core
SCALE = 1.0 / np.sqrt(DK)

_compiled = {}


def _build():
    import concourse.bacc as bacc
    import concourse.tile as tile
    from concourse import mybir

    f32 = mybir.dt.float32
    bf16 = mybir.dt.bfloat16
    Exp = mybir.ActivationFunctionType.Exp
    mult = mybir.AluOpType.mult

    nc = bacc.Bacc("TRN2", target_bir_lowering=False, debug=False,
                   enable_asserts=True, num_devices=NCORES)

    NDT = D // 128   # 8 d-tiles
    NST = S // 128   # 16 s-tiles (j tiles)
    NSB = S // 512   # 4 s-blocks (i blocks)

    # inputs pre-relayouted on host: x pieces are [128, 8(dt), cols] with the
    # partition dim first so one DMA is 128 contiguous descriptors.
    XQW = [256, 256, 512, 1024]
    XKW = [256, 256, 512, 512, 512]
    xqd = [nc.dram_tensor(f"xq{i}", (128, NDT, w), bf16, kind="ExternalInput")
           for i, w in enumerate(XQW)]
    xkd = [nc.dram_tensor(f"xk{i}", (128, NDT, w), bf16, kind="ExternalInput")
           for i, w in enumerate(XKW)]
    xvd = [nc.dram_tensor(f"xv{i}", shp, bf16, kind="ExternalInput")
           for i, shp in enumerate([(128, NDT, 1024), (128, NDT, 1024)])]
    wq = nc.dram_tensor("wq", (128, NDT, F), bf16, kind="ExternalInput")
    wk = nc.dram_tensor("wk", (128, NDT, F), bf16, kind="ExternalInput")
    wv = nc.dram_tensor("wv", (128, NDT, F), bf16, kind="ExternalInput")
    wo = nc.dram_tensor("wo", (128, 2, D), bf16, kind="ExternalInput")
    bq = nc.dram_tensor("bq", (128, 2), f32, kind="ExternalInput")  # bias, f-tiled
    bk = nc.dram_tensor("bk", (128, 2), f32, kind="ExternalInput")
    bv = nc.dram_tensor("bv", (1, F), f32, kind="ExternalInput")
    out = nc.dram_tensor("out", (128, NSB, NDT, 512), bf16, kind="ExternalOutput")

    with tile.TileContext(nc) as tc:
        import contextlib
        with contextlib.ExitStack() as ctx:
            consts = ctx.enter_context(tc.tile_pool(name="consts", bufs=1))
            atp = ctx.enter_context(tc.tile_pool(name="atp", bufs=10))
            acts = ctx.enter_context(tc.tile_pool(name="acts", bufs=1))
            ostage = ctx.enter_context(tc.tile_pool(name="ostage", bufs=3))
            small = ctx.enter_context(tc.tile_pool(name="small", bufs=1))
            ps = ctx.enter_context(tc.tile_pool(name="ps", bufs=1, space="PSUM"))

            # ---- resident input/weight tiles (one per dram piece) ----
            xq_sb = [acts.tile([128, NDT, w], bf16, tag=f"xq{i}", name=f"xq{i}")
                     for i, w in enumerate(XQW)]
            xk_sb = [acts.tile([128, NDT, w], bf16, tag=f"xk{i}", name=f"xk{i}")
                     for i, w in enumerate(XKW)]
            xv_sb = [acts.tile([128, NDT, 1024], bf16, tag=f"xv{i}", name=f"xv{i}")
                     for i in range(2)]

            def x_ap(ts, dt, c0, w):
                # columns [c0, c0+w) of d-tile dt across the piece tiles
                off = 0
                for t in ts:
                    n = t.shape[2]
                    if c0 < off + n:
                        return t[:, dt, c0 - off:c0 - off + w]
                    off += n
                raise AssertionError

            def xv_ap(dt, st):
                t, r = divmod(st, 8)
                return xv_sb[t][:, dt, r * 128:(r + 1) * 128]

            wq_sb = consts.tile([128, NDT, F], bf16, tag="wq")
            wk_sb = consts.tile([128, NDT, F], bf16, tag="wk")
            wv_sb = consts.tile([128, NDT, F], bf16, tag="wv")
            wo_sb = consts.tile([128, 2, D], bf16, tag="wo")
            bq_sb = consts.tile([128, 2], f32, tag="bq")
            bk_sb = consts.tile([128, 2], f32, tag="bk")
            bv_sb = consts.tile([128, F], f32, tag="bv")

            # persistent activations
            qh_t = [[acts.tile([128, 512], bf16, tag=f"qh{ft}{sb}", name=f"qh{ft}{sb}")
                     for sb in range(NSB)] for ft in range(2)]
            kh_t = [[acts.tile([128, 512], bf16, tag=f"kh{ft}{sb}", name=f"kh{ft}{sb}")
                     for sb in range(NSB)] for ft in range(2)]
            # vh: [s, h, c+1] with ones column at c=64 (PV denominator trick)
            vh_t = [acts.tile([128, HPG, DK + 1], bf16, tag=f"vh{st}", name=f"vh{st}")
                    for st in range(NST)]
            for st in range(NST):
                nc.vector.memset(vh_t[st][:, :, DK:DK + 1], 1.0)
            # y: normalized attention output, [f, s] per (ft, sb)
            y_t = [[acts.tile([128, 512], bf16, tag=f"y{ft}{sb}", name=f"y{ft}{sb}")
                    for sb in range(NSB)] for ft in range(2)]

            # ---- input DMAs: whole-piece transfers issued in priority
            # order (queues process descriptors FIFO, so issue order is
            # arrival order; explicit dep-chaining costs ~5us per link) ----
            def dma_in(dst, src):
                return nc.sync.dma_start(dst, src)

            dma_in(wk_sb[:], wk.ap())
            dma_in(xk_sb[0][:], xkd[0].ap())
            dma_in(wq_sb[:], wq.ap())
            dma_in(xq_sb[0][:], xqd[0].ap())
            nc.sync.dma_start(bk_sb[:], bk.ap()[:])
            nc.sync.dma_start(bq_sb[:], bq.ap()[:])
            dma_in(xk_sb[1][:], xkd[1].ap())
            dma_in(xq_sb[1][:], xqd[1].ap())
            nc.sync.dma_start(bv_sb[:], bv.ap().to_broadcast((128, F)))
            dma_in(xk_sb[2][:], xkd[2].ap())
            dma_in(xk_sb[3][:], xkd[3].ap())
            dma_in(xk_sb[4][:], xkd[4].ap())
            dma_in(wv_sb[:], wv.ap())
            dma_in(xv_sb[0][:], xvd[0].ap())
            dma_in(xq_sb[2][:], xqd[2].ap())
            dma_in(xv_sb[1][:], xvd[1].ap())
            dma_in(xq_sb[3][:], xqd[3].ap())
            dma_in(wo_sb[:], wo.ap())

            # ---- PE warmup: keep HAM at 8/8 through the DMA lead-in so the
            # first projections run at 2.4 GHz instead of 1.2 ----
            wupa = consts.tile([128, 32], bf16, tag="wup")
            nc.vector.memset(wupa[:], 0.0)
            wups = ps.tile([128, 512], f32, tag="w1", bufs=4, name="wups")

            def warm(n):
                for _ in range(n):
                    nc.tensor.matmul(wups[0:32, 0:32], wupa[:, 0:32],
                                     wupa[:, 0:32], start=True, stop=True)
            warm(150)

            # ---- background PE units (woven into the attention steps) ----
            _uid = [0]

            def proj_unit(w_sb, b_sb, xts, dst, ft, sb, off=0, w=512):
                _uid[0] += 1
                acc = ps.tile([128, 512], f32, tag="w1", bufs=4,
                              name=f"pj{_uid[0]}")
                # split the column window at x-piece boundaries
                bounds, c = [], 0
                for t in xts:
                    c += t.shape[2]
                    bounds.append(c)
                c0, wins = sb * 512 + off, []
                end = c0 + w
                while c0 < end:
                    nxt = min([b for b in bounds if b > c0] + [end])
                    wins.append((c0, min(nxt, end)))
                    c0 = min(nxt, end)
                for (a, b_) in wins:
                    oa = a - (sb * 512)
                    for dt in range(NDT):
                        nc.tensor.matmul(
                            acc[:, oa:oa + b_ - a],
                            w_sb[:, dt, ft * 128:(ft + 1) * 128],
                            x_ap(xts, dt, a, b_ - a),
                            start=(dt == 0), stop=(dt == NDT - 1),
                        )
                nc.vector.tensor_scalar_add(dst[ft][sb][:, off:off + w],
                                            acc[:, off:off + w],
                                            b_sb[:, ft:ft + 1])

            def v_unit(g):
                accs = [ps.tile([128, 512], f32, tag="w1", bufs=4,
                                name=f"vps{g}{j}") for j in range(2)]
                for dt in range(NDT):
                    for j in range(2):
                        st = 2 * g + j
                        nc.tensor.matmul(
                            accs[j][:, 0:F],
                            xv_ap(dt, st),
                            wv_sb[:, dt, :],
                            start=(dt == 0), stop=(dt == NDT - 1),
                        )
                for j in range(2):
                    st = 2 * g + j
                    nc.vector.tensor_tensor(
                        vh_t[st][:, :, 0:DK],
                        accs[j][:, 0:F].rearrange("p (h c) -> p h c", h=HPG),
                        bv_sb[:].rearrange("p (h c) -> p h c", h=HPG),
                        mybir.AluOpType.add,
                    )

            # ---- attention chunk helpers (per (ft, ib) step, jc chunks) ----
            def scores_chunk(pr, ib, jc, halves=None):
                # halves: list of (q0, q1) column windows; each gets its own
                # exp call (used to start ACT before the full qh is ready)
                ft = pr
                at = atp.tile([128, 4, 512], bf16, tag="at", name=f"at{pr}{ib}{jc}")
                for jj in range(2):
                    jt = jc * 2 + jj
                    sc = ps.tile([128, 2, 512], f32, tag="w2", bufs=2, name="sc")
                    for (q0, q1) in (halves or [(0, 512)]):
                        for hh in range(2):
                            base = hh * 64
                            nc.tensor.matmul(
                                sc[:, hh, q0:q1],
                                kh_t[ft][jt // 4][base:base + 64,
                                                  (jt % 4) * 128:(jt % 4 + 1) * 128],
                                qh_t[ft][ib][base:base + 64, q0:q1],
                                start=True, stop=True,
                                tile_position=(base, 0),
                            )
                        nc.scalar.activation(
                            at[:, jj * 2:jj * 2 + 2, q0:q1],
                            sc[:, :, q0:q1],
                            Exp, scale=float(SCALE),
                        )
                return at

            def pv_chunk(pr, pv_ps, at, jc):
                for hh in range(2):
                    h = 2 * pr + hh
                    for jj in range(2):
                        jt = 2 * jc + jj
                        nc.tensor.matmul(
                            pv_ps[hh][0:DK + 1, :],
                            vh_t[jt][:, h, :],
                            at[:, 2 * jj + hh, :],
                            start=(jt == 0), stop=(jt == NST - 1),
                        )

            def finish_ib(pr, ib, pv_ps):
                # hh0/hh1 chains interleaved so DVE and GPSIMD pipeline
                ft = pr
                dens, recs, rbs = [], [], []
                for hh in range(2):
                    den = small.tile([1, 512], f32, tag=f"den{hh}")
                    nc.vector.tensor_copy(den[:], pv_ps[hh][DK:DK + 1, :])
                    dens.append(den)
                for hh in range(2):
                    rec = small.tile([1, 512], f32, tag=f"rec{hh}")
                    nc.vector.reciprocal_approx_fast(rec[:], dens[hh][:])
                    recs.append(rec)
                for hh in range(2):
                    rb = small.tile([64, 512], f32, tag=f"rb{hh}")
                    nc.gpsimd.partition_broadcast(rb[:], recs[hh][:])
                    rbs.append(rb)
                for hh in range(2):
                    nc.vector.tensor_tensor(
                        y_t[ft][ib][hh * 64:hh * 64 + 64, :],
                        pv_ps[hh][0:DK, :],
                        rbs[hh][:],
                        mult,
                    )

            # ---- output projection: one unit = 2 et tiles + 1 out DMA ----
            def outproj_unit(sb, ep):
                o_sb = ostage.tile([128, 2, 512], bf16, tag="ost",
                                   name=f"os{ep}{sb}")
                for i in range(2):
                    et = 2 * ep + i
                    po = ps.tile([128, 512], f32, tag="w1", bufs=4,
                                 name=f"po{et}{sb}")
                    for ft in range(2):
                        nc.tensor.matmul(
                            po[:],
                            wo_sb[:, ft, et * 128:(et + 1) * 128],
                            y_t[ft][sb][:],
                            start=(ft == 0), stop=(ft == 1),
                        )
                    nc.vector.tensor_copy(o_sb[:, i, :], po[:])
                nc.sync.dma_start(
                    out.ap()[:, sb, 2 * ep:2 * ep + 2, :],
                    o_sb[:],
                )

            # ---- the woven schedule ----
            # stage 1: half-width lead-in — kh/qh for queries/keys 0:256
            # only (the 256-wide first DMA pieces), so the PE starts ~5us
            # earlier; the first two scores chunks are emitted query-split.
            proj_unit(wk_sb, bk_sb, xk_sb, kh_t, 0, 0, 0, 256)
            warm(80)   # bridge the xq0a DMA wait so HAM stays at 8/8
            proj_unit(wq_sb, bq_sb, xq_sb, qh_t, 0, 0, 0, 256)
            warm(40)

            def first_chunk():
                # jc0 of step (0,0), query-split so scoring starts on the
                # 256-wide lead DMA pieces; the 256:512 projections are
                # emitted between the two halves (PE queue is in-order).
                at = atp.tile([128, 4, 512], bf16, tag="at", name="at000")
                scs = [ps.tile([128, 2, 512], f32, tag="w2", bufs=2,
                               name="sc") for _ in range(2)]

                def mmhalf(jj, q0, q1):
                    for hh in range(2):
                        base = hh * 64
                        nc.tensor.matmul(
                            scs[jj][:, hh, q0:q1],
                            kh_t[0][0][base:base + 64, jj * 128:(jj + 1) * 128],
                            qh_t[0][0][base:base + 64, q0:q1],
                            start=True, stop=True,
                            tile_position=(base, 0),
                        )
                    nc.scalar.activation(
                        at[:, jj * 2:jj * 2 + 2, q0:q1],
                        scs[jj][:, :, q0:q1],
                        Exp, scale=float(SCALE),
                    )
                mmhalf(0, 0, 256)
                mmhalf(1, 0, 256)
                proj_unit(wq_sb, bq_sb, xq_sb, qh_t, 0, 0, 256, 256)
                mmhalf(0, 256, 512)
                mmhalf(1, 256, 512)
                proj_unit(wk_sb, bk_sb, xk_sb, kh_t, 0, 0, 256, 256)
                return at

            # background units per (step, jc-slot), placed after both their
            # DMA piece lands (see issue order) and before their consumer.
            # v(g) feeds pv chunk g of the NEXT step; kh(0,sb) feeds scores
            # jc=2sb of step 0; qh(ft,ib) feeds step (4ft+ib)'s scores;
            # op(sb,ep) = output projection units, after finish of (1,sb).
            slots = {
                (0, 1): [("kh", 0, 1)],
                (0, 3): [("kh", 0, 2)],
                (0, 4): [("kh", 0, 3)],
                (0, 5): [("v", 0)],
                (0, 6): [("v", 1)],
                (0, 7): [("qh", 0, 1)],
                (1, 0): [("v", 2)],
                (1, 1): [("v", 3)],
                (1, 2): [("v", 4)],
                (1, 3): [("v", 5)],
                (1, 4): [("v", 6)],
                (1, 5): [("v", 7)],
                (1, 7): [("qh", 0, 2)],
                (2, 1): [("kh", 1, 0)],
                (2, 2): [("qh", 0, 3)],
                (2, 4): [("kh", 1, 1)],
                (3, 1): [("kh", 1, 2)],
                (3, 2): [("kh", 1, 3)],
                (3, 4): [("qh", 1, 0)],
                (4, 1): [("qh", 1, 1)],
                (5, 1): [("qh", 1, 2)],
                (6, 1): [("qh", 1, 3)],
                (6, 2): [("op", 0, 0)], (6, 3): [("op", 0, 1)],
                (6, 4): [("op", 0, 2)], (6, 5): [("op", 0, 3)],
                (7, 2): [("op", 1, 0)], (7, 3): [("op", 1, 1)],
                (7, 4): [("op", 1, 2)], (7, 5): [("op", 1, 3)],
            }

            def run_unit(u):
                if u[0] == "kh":
                    proj_unit(wk_sb, bk_sb, xk_sb, kh_t, u[1], u[2])
                elif u[0] == "qh":
                    proj_unit(wq_sb, bq_sb, xq_sb, qh_t, u[1], u[2])
                elif u[0] == "v":
                    v_unit(u[1])
                elif u[0] == "op":
                    outproj_unit(u[1], u[2])

            # software pipeline: the previous step's pv chunk jc-1 runs at
            # slot jc; its last chunk + finish run after the NEXT step's
            # first scores chunk so the step boundary never blocks ACT.
            seq = [(0, 0), (0, 1), (0, 2), (0, 3), (1, 0), (1, 1), (1, 2), (1, 3)]
            prev = None   # (pr, ib, pv_ps, at_list) - one step behind
            prev2 = None  # two steps behind, needs last chunk + finish
            for si, (pr, ib) in enumerate(seq):
                at_list = []
                ppv = None
                for jc in range(NST // 2):
                    if si == 0 and jc == 0:
                        at_list.append(first_chunk())
                        continue
                    at_list.append(scores_chunk(pr, ib, jc))
                    if jc == 0:
                        if prev2 is not None:
                            pv_chunk(prev2[0], prev2[2], prev2[3][7], 7)
                            finish_ib(prev2[0], prev2[1], prev2[2])
                    else:
                        if prev is not None:
                            if jc == 1:
                                ppv = [ps.tile([128, 512], f32, tag="w1", bufs=4,
                                               name=f"pv{prev[0]}{prev[1]}_{i}")
                                       for i in range(2)]
                                prev = (prev[0], prev[1], ppv, prev[3])
                            pv_chunk(prev[0], prev[2], prev[3][jc - 1], jc - 1)
                    for u in slots.get((si, jc), ()):
                        run_unit(u)
                prev2 = prev
                prev = (pr, ib, None, at_list)

            # tail: finish step 6's pipeline, then drain step 7's PV with the
            # sb2 output projection interleaved; finishes overlap on DVE.
            pv_chunk(prev2[0], prev2[2], prev2[3][7], 7)
            finish_ib(prev2[0], prev2[1], prev2[2])
            lpv = [ps.tile([128, 512], f32, tag="w1", bufs=4, name=f"pvlast{i}")
                   for i in range(2)]
            for jc in range(NST // 2):
                pv_chunk(prev[0], lpv, prev[3][jc], jc)
                if jc >= 4:
                    outproj_unit(2, jc - 4)   # y[*][2] ready via finish above
            finish_ib(prev[0], prev[1], lpv)   # DVE, overlaps outproj(2) PE
            for ep in range(4):
                outproj_unit(3, ep)

    nc.compile()
    return nc


def _get_nc():
    if "nc" not in _compiled:
        _compiled["nc"] = _build()
    return _compiled["nc"]


def kernel(q, k, v, Wq, bq, Wk, bk, Wv, bv, Wo, bo):
    outp, _ = _run(q, k, v, Wq, bq, Wk, bk, Wv, bv, Wo, bo)
    return outp


def _x_pieces(xT, bf, widths):
    x3 = np.transpose(xT.reshape(8, 128, S), (1, 0, 2))  # [128, 8, S]
    out, c = [], 0
    for w in widths:
        out.append(np.ascontiguousarray(x3[:, :, c:c + w]).astype(bf))
        c += w
    return out


def _v_pieces(xT, bf):
    x3 = np.transpose(xT.reshape(8, 128, S), (1, 0, 2))
    return [np.ascontiguousarray(x3[:, :, 0:1024]).astype(bf),
            np.ascontiguousarray(x3[:, :, 1024:2048]).astype(bf)]


def _w_relayout(wT, bf):
    # wT: [D, F] -> [128, 8, F]
    return np.ascontiguousarray(
        np.transpose(wT.reshape(8, 128, F), (1, 0, 2))).astype(bf)


def _run(q, k, v, Wq, bq, Wk, bk, Wv, bv, Wo, bo, **run_kwargs):
    from concourse.bass_utils import run_bass_kernel_spmd

    nc = _get_nc()

    q = np.asarray(q, np.float32)
    k = np.asarray(k, np.float32)
    v = np.asarray(v, np.float32)
    Wq = np.asarray(Wq, np.float32)
    Wk = np.asarray(Wk, np.float32)
    Wv = np.asarray(Wv, np.float32)
    Wo = np.asarray(Wo, np.float32)
    bq = np.asarray(bq, np.float32)
    bk = np.asarray(bk, np.float32)
    bv = np.asarray(bv, np.float32)
    bo = np.asarray(bo, np.float32)

    import ml_dtypes
    bf = ml_dtypes.bfloat16
    xqP = [_x_pieces(np.ascontiguousarray(q[b].T), bf, [256, 256, 512, 1024]) for b in range(B)]
    xkP = [_x_pieces(np.ascontiguousarray(k[b].T), bf, [256, 256, 512, 512, 512]) for b in range(B)]
    xvP = [_v_pieces(np.ascontiguousarray(v[b].T), bf) for b in range(B)]

    in_maps = []
    for c in range(NCORES):
        b, hg = divmod(c, HG)
        rows = slice(hg * F, (hg + 1) * F)
        woT = np.ascontiguousarray(Wo[:, rows].T)  # [F, D]
        wo_r = np.ascontiguousarray(
            np.transpose(woT.reshape(2, 128, D), (1, 0, 2))).astype(bf)
        m = {
            "wq": _w_relayout(np.ascontiguousarray(Wq[rows].T), bf),
            "wk": _w_relayout(np.ascontiguousarray(Wk[rows].T), bf),
            "wv": _w_relayout(np.ascontiguousarray(Wv[rows].T), bf),
            "wo": wo_r,
            "bq": np.ascontiguousarray(bq[rows].reshape(2, 128).T),
            "bk": np.ascontiguousarray(bk[rows].reshape(2, 128).T),
            "bv": np.ascontiguousarray(bv[rows].reshape(1, F)),
        }
        for i in range(4):
            m[f"xq{i}"] = xqP[b][i]
        for i in range(5):
            m[f"xk{i}"] = xkP[b][i]
        for i in range(2):
            m[f"xv{i}"] = xvP[b][i]
        in_maps.append(m)

    res = run_bass_kernel_spmd(nc, in_maps, core_ids=list(range(NCORES)), **run_kwargs)

    outp = np.empty((B, S, D), np.float32)
    for b in range(B):
        acc = res.results[b * HG]["out"].astype(np.float32)
        for hg in range(1, HG):
            acc = acc + res.results[b * HG + hg]["out"].astype(np.float32)
        # [128, 4, 8, 512] -> [D, S] -> [S, D]
        full = np.transpose(acc, (2, 0, 1, 3)).reshape(D, S)
        outp[b] = full.T + bo[None, :]
    return outp, res
